# revision 1
# baseline (speedup 1.0000x reference)
"""Trainium2 Bass kernel for a 3-layer binarized CNN.

Network (reference):
    x  : [32, 3, 512, 512] fp32
    l1 : clip(conv(x, sign(w1)))            -> [32,16,510,510]
    l2 : clip(conv(sign(l1), sign(w2)))     -> [32,23,508,508]
    l3 : clip(conv(sign(l2), sign(w3)))     -> [32,2,506,506]
    out: l3.reshape(32, -1)

Strategy (pure data parallel, 4 images per NeuronCore x 8 cores):
  * All convs are Toeplitz-stationary matmuls on the tensor engine.
    The contraction (partition) axis holds a window of image ROWS
    interleaved with input channels; the moving axis streams image
    columns.  The 3 kernel taps along the column axis are 3
    PSUM-accumulated matmuls whose rhs is shifted in the free dim.
  * Layer 1 only matters through the SIGN of its output, so it needs
    ~fp32 accuracy: the fp32 input is split x = hi + mid + lo into
    three fp16 terms folded into the contraction axis (weights are
    +-1, exact in fp16) -> error ~2^-33, below fp32's own 2^-24.
  * Layers 2/3 have +-1 x +-1 products with small integer sums: fp8
    inputs with fp32 PSUM accumulation are bit-exact, and fp8
    DoubleRow perf mode (2 MACs/cell/cycle) is used with channel
    pairs interleaved in the free dimension.
  * sign()/clip() of the integer-valued PSUM are both exactly
    clip(x,-1,1) = Sign(x); they run on the idle Scalar engine.
  * Intermediates bounce through DRAM in [row, channel, col] layout
    so every DMA moves long contiguous runs.
"""

import numpy as np
import ml_dtypes

import concourse.bacc as bacc
import concourse.mybir as mybir
import concourse.tile as tile
from concourse.bass_utils import run_bass_kernel_spmd

F32 = mybir.dt.float32
F16 = mybir.dt.float16
F8 = mybir.dt.float8e4
NP_F8 = ml_dtypes.float8_e4m3
ALU = mybir.AluOpType
DR = mybir.MatmulPerfMode.DoubleRow
SIGN = mybir.ActivationFunctionType.Sign

N_CORES = 8

AL1, AO1 = 10, 8     # L1: rows window / rows out per block
AL2, AO2 = 7, 5      # L2
AL3, AO3 = 32, 30    # L3
C1, C2, C3 = 3, 16, 23
O1, O2, O3 = 16, 23, 2


def _toeplitz_weights(w1, w2, w3):
    """Build the stationary Toeplitz matrices (host side)."""
    s1 = np.sign(w1).astype(np.float32)  # [16,3,3,3]
    s2 = np.sign(w2).astype(np.float32)  # [23,16,3,3]
    s3 = np.sign(w3).astype(np.float32)  # [2,23,3,3]

    # T1[(term*32 + al*3 + c), dx, (aol*16 + o)]; spacer rows zero.
    t1 = np.zeros((96, 3, AO1 * O1), np.float32)
    for al in range(AL1):
        for aol in range(AO1):
            dy = al - aol
            if 0 <= dy <= 2:
                for c in range(C1):
                    for t in range(3):
                        for o in range(O1):
                            t1[t * 32 + al * 3 + c, :, aol * 16 + o] = s1[o, c, dy, :]
    # T2[(al*8 + cp), dx, codd, (aol*23 + o)] fp8 DoubleRow pairs, M pad 128
    t2 = np.zeros((56, 3, 2, 128), np.float32)
    for al in range(AL2):
        for aol in range(AO2):
            dy = al - aol
            if 0 <= dy <= 2:
                for c in range(C2):
                    for o in range(O2):
                        t2[al * 8 + c // 2, :, c % 2, aol * 23 + o] = s2[o, c, dy, :]
    # T3[(al*4 + cp), cc, dx, codd, (aol*2 + o)] fp8 DoubleRow, M pad 64
    t3 = np.zeros((128, 3, 3, 2, 64), np.float32)
    for al in range(AL3):
        for aol in range(AO3):
            dy = al - aol
            if 0 <= dy <= 2:
                for cc in range(3):
                    for cl in range(8):
                        c = cc * 8 + cl
                        if c < C3:
                            for o in range(O3):
                                t3[al * 4 + cl // 2, cc, :, cl % 2, aol * 2 + o] = (
                                    s3[o, c, dy, :]
                                )
    return (
        t1.reshape(96, 3 * 128).astype(np.float16),
        t2.reshape(56, 3 * 256).astype(NP_F8),
        t3.reshape(128, 9 * 128).astype(NP_F8),
    )


def _build_program(n_img, A, B, layers=(1, 2, 3)):
    """Emit the per-core SPMD Bass program (all 3 layers, n_img images)."""
    nblk1 = -(-(A - 2) // AO1)          # L1 row-blocks (of 8), padded
    nch1 = -(-nblk1 // 2)
    a_pad = 16 * nch1 + 2               # input rows needed (zero padded)
    a1 = AO1 * nblk1                    # s1 row count (incl. garbage tail)
    nblk2 = -(-(A - 4) // AO2)
    nblk3 = -(-(A - 6) // AO3)
    s2a = max(AO2 * nblk2, AO3 * (nblk3 - 1) + AL3)  # s2 rows incl. zero pad
    n1, n2, n3 = B - 2, B - 4, B - 6

    assert AL2 + AO2 * (nblk2 - 1) <= a1, "L2 reads past s1"

    nc = bacc.Bacc("TRN2", target_bir_lowering=False, debug=False)

    xt = nc.dram_tensor("xt", [n_img, 3, a_pad, B], F32, kind="ExternalInput")
    t1w = nc.dram_tensor("t1w", [96, 3 * 128], F16, kind="ExternalInput")
    t2w = nc.dram_tensor("t2w", [56, 3 * 256], F8, kind="ExternalInput")
    t3w = nc.dram_tensor("t3w", [128, 9 * 128], F8, kind="ExternalInput")
    outp = nc.dram_tensor(
        "outp", [n_img, AO3 * nblk3, 2, n3], F8, kind="ExternalOutput"
    )
    s1d = [
        nc.dram_tensor(f"s1_{i}", [a1, 16, n1], F8, kind="Internal")
        for i in range(n_img)
    ]
    s2d = [
        nc.dram_tensor(f"s2_{i}", [s2a, 24, n2], F8, kind="Internal")
        for i in range(n_img)
    ]

    with tile.TileContext(nc) as tc:
        with (
            tc.tile_pool(name="const", bufs=1) as cpool,
            tc.tile_pool(name="l1x", bufs=2) as p1x,
            tc.tile_pool(name="l1s", bufs=4) as p1s,
            tc.tile_pool(name="l2", bufs=4) as p2,
            tc.tile_pool(name="l3", bufs=8) as p3,
            tc.tile_pool(name="ps1", bufs=3, space="PSUM") as ps1p,
            tc.tile_pool(name="ps2", bufs=2, space="PSUM") as ps2p,
            tc.tile_pool(name="ps3", bufs=2, space="PSUM") as ps3p,
        ):
            t1sb = cpool.tile([96, 3 * 128], F16)
            t2sb = cpool.tile([56, 3 * 256], F8)
            t3sb = cpool.tile([128, 9 * 128], F8)
            ztile = cpool.tile([128, B], F8)
            nc.sync.dma_start(t1sb[:], t1w.ap()[:])
            nc.sync.dma_start(t2sb[:], t2w.ap()[:])
            nc.sync.dma_start(t3sb[:], t3w.ap()[:])
            nc.vector.memset(ztile[:], 0.0)
            # persistent L1 rhs ring: hi rows 0..29, mid 32..61, lo 64..93.
            NRHS = 3
            rhs_ring = []
            for ri in range(NRHS):
                rt = cpool.tile([96, B], F16, name=f"rhs1_{ri}")
                nc.vector.memset(rt[:], 0.0)
                rhs_ring.append(rt)

            for img in range(n_img):
                s1, s2 = s1d[img].ap(), s2d[img].ap()
                # ---- zero pads of s2: channel-23 plane + tail rows ----
                for r in range(0, s2a, 128):
                    cnt = min(128, s2a - r)
                    nc.sync.dma_start(s2[r : r + cnt, 23, :], ztile[:cnt, :n2])
                for a in range(AO2 * nblk2, s2a):
                    nc.sync.dma_start(s2[a, :, :], ztile[:24, :n2])

                # ---------------- layer 1 ----------------
                # row-groups of up to 15 blocks; the fp16 3-way split runs
                # once per group with rows on partitions (full DVE lanes),
                # then per-block rhs assembly is pure SBUF->SBUF DMA.
                for g0 in range(0, nblk1 if 1 in layers else 0, 15):
                    g1 = min(g0 + 15, nblk1)
                    r0 = 8 * g0
                    rcnt = 8 * (g1 - 1 - g0) + AL1
                    xg = p1x.tile([122, 3 * B], F32, tag="xg")
                    nc.sync.dma_start(
                        xg[0:rcnt, :],
                        xt.ap()[img, :, r0 : r0 + rcnt, :].transpose([1, 0, 2]),
                    )
                    hig = p1x.tile([122, 3 * B], F16, tag="hig")
                    dg = p1x.tile([122, 3 * B], F32, tag="dg")
                    mig = p1x.tile([122, 3 * B], F16, tag="mig")
                    log_ = p1x.tile([122, 3 * B], F16, tag="log")
                    nc.vector.tensor_copy(hig[0:rcnt, :], xg[0:rcnt, :])
                    nc.vector.scalar_tensor_tensor(
                        dg[0:rcnt, :], xg[0:rcnt, :], 1.0, hig[0:rcnt, :],
                        op0=ALU.mult, op1=ALU.subtract,
                    )
                    nc.vector.tensor_copy(mig[0:rcnt, :], dg[0:rcnt, :])
                    nc.vector.scalar_tensor_tensor(
                        log_[0:rcnt, :], dg[0:rcnt, :], 1.0, mig[0:rcnt, :],
                        op0=ALU.mult, op1=ALU.subtract,
                    )
                    for blk in range(g0, g1):
                        a0 = 8 * blk
                        lr = a0 - r0
                        rhs16 = rhs_ring[blk % NRHS]
                        for ti, tsrc in enumerate((hig, mig, log_)):
                            nc.sync.dma_start(
                                rhs16[32 * ti : 32 * ti + 30, :],
                                tsrc[lr : lr + AL1, :],
                            )
                        ps = ps1p.tile([128, n1], F32, tag="ps1")
                        for dx in range(3):
                            nc.tensor.matmul(
                                ps[:],
                                t1sb[:, 128 * dx : 128 * dx + 128],
                                rhs16[:, dx : dx + n1],
                                start=(dx == 0),
                                stop=(dx == 2),
                            )
                        pos16 = p1s.tile([128, n1], F16, tag="pos16")
                        nc.vector.tensor_scalar(
                            pos16[:], ps[:], 0.0, None, op0=ALU.is_gt
                        )
                        sg8 = p1s.tile([128, n1], F8, tag="sg8")
                        nc.vector.tensor_scalar(
                            sg8[:], pos16[:], 2.0, -1.0, op0=ALU.mult, op1=ALU.add
                        )
                        nc.sync.dma_start(s1[a0 : a0 + 8, :, :], sg8[:])

                # ---------------- layer 2 (fp8 DoubleRow) ----------------
                for b in range(nblk2 if 2 in layers else 0):
                    rhs8 = p2.tile([56, 2 * 512], F8, tag="rhs8")
                    r3 = rhs8[:].rearrange("k (t h) -> k t h", t=2)
                    nc.sync.dma_start(r3[:, :, 0:n1], s1[5 * b : 5 * b + 7, :, :])
                    ps = ps2p.tile([115, n2], F32, tag="ps2")
                    for dx in range(3):
                        nc.tensor.matmul(
                            ps[:],
                            t2sb[:, 256 * dx : 256 * dx + 256].rearrange(
                                "k (t m) -> k t m", t=2
                            )[:, :, 0:115],
                            r3[:, :, dx : dx + n2],
                            start=(dx == 0),
                            stop=(dx == 2),
                            perf_mode=DR,
                        )
                    sg2 = p2.tile([115, n2], F8, tag="sg2")
                    nc.scalar.activation(sg2[:], ps[:], SIGN)
                    nc.sync.dma_start(s2[5 * b : 5 * b + 5, 0:23, :], sg2[:])

                # ---------------- layer 3 (fp8 DoubleRow) ----------------
                for bb in range(nblk3 if 3 in layers else 0):
                    rts = []
                    for cc in range(3):
                        rt = p3.tile([128, 2 * 512], F8, tag="rhs3")
                        nc.sync.dma_start(
                            rt[:].rearrange("k (t h) -> k t h", t=2)[:, :, 0:n2],
                            s2[30 * bb : 30 * bb + 32, 8 * cc : 8 * cc + 8, :],
                        )
                        rts.append(rt)
                    ps = ps3p.tile([60, n3], F32, tag="ps3")
                    for cc in range(3):
                        for dx in range(3):
                            nc.tensor.matmul(
                                ps[:],
                                t3sb[
                                    :, 128 * (cc * 3 + dx) : 128 * (cc * 3 + dx) + 128
                                ].rearrange("k (t m) -> k t m", t=2)[:, :, 0:60],
                                rts[cc][:].rearrange("k (t h) -> k t h", t=2)[
                                    :, :, dx : dx + n3
                                ],
                                start=(cc == 0 and dx == 0),
                                stop=(cc == 2 and dx == 2),
                                perf_mode=DR,
                            )
                    oc = p3.tile([60, n3], F8, tag="oc")
                    nc.scalar.activation(oc[:], ps[:], SIGN)
                    nc.sync.dma_start(
                        outp.ap()[img, 30 * bb : 30 * bb + 30, :, :], oc[:]
                    )

    nc.compile()
    return nc


_CACHE = {}


def _get_program(n_img, A, B):
    key = (n_img, A, B)
    if key not in _CACHE:
        _CACHE[key] = _build_program(n_img, A, B)
    return _CACHE[key]


def make_in_maps(x, w1, w2, w3, n_cores=N_CORES, a_pad=None):
    """x: [N,3,A,B] fp32 -> list of per-core input maps."""
    n, _, A, B = x.shape
    per = n // n_cores
    nblk1 = -(-(A - 2) // AO1)
    nch1 = -(-nblk1 // 2)
    if a_pad is None:
        a_pad = 16 * nch1 + 2
    t1, t2, t3 = _toeplitz_weights(
        np.asarray(w1, np.float32), np.asarray(w2, np.float32), np.asarray(w3, np.float32)
    )
    xp = np.zeros((n, 3, a_pad, B), np.float32)
    xp[:, :, :A, :] = np.asarray(x, np.float32)
    maps = []
    for i in range(n_cores):
        maps.append(
            {
                "xt": np.ascontiguousarray(xp[per * i : per * (i + 1)]),
                "t1w": t1,
                "t2w": t2,
                "t3w": t3,
            }
        )
    return maps


last_results = None


def kernel(inputs, w1, w2, w3):
    global last_results
    x = np.asarray(inputs, np.float32)
    n, _, A, B = x.shape
    per = n // N_CORES
    nc = _get_program(per, A, B)
    maps = make_in_maps(x, w1, w2, w3)
    res = run_bass_kernel_spmd(nc, maps, core_ids=list(range(N_CORES)))
    last_results = res
    a3, b3 = A - 6, B - 6
    out = np.empty((n, 2, a3, b3), np.float32)
    for i, r in enumerate(res.results):
        o = r["outp"][:, :a3, :, :].astype(np.float32)  # [per, a3, 2, b3]
        out[per * i : per * (i + 1)] = o.transpose(0, 2, 1, 3)
    return out.reshape(n, -1)



# revision 4
# speedup vs baseline: 1.0563x; 1.0563x over previous
"""Trainium2 Bass kernel for a 3-layer binarized CNN.

Network (reference):
    x  : [32, 3, 512, 512] fp32
    l1 : clip(conv(x, sign(w1)))            -> [32,16,510,510]
    l2 : clip(conv(sign(l1), sign(w2)))     -> [32,23,508,508]
    l3 : clip(conv(sign(l2), sign(w3)))     -> [32,2,506,506]
    out: l3.reshape(32, -1)

The end-to-end wall time is dominated by the ~70-80 MB/s axon tunnel
(plus ~50-80 ms fixed cost per transferred array), so the kernel
minimizes both bytes and array count on the wire:

  * Input is sent as an exact 21-bit fixed-point code (3 B/elem instead
    of 4): t = x*a + 3*2^22 in fp32 makes the mantissa a fixed-point
    integer m = round(x*a) + 2^22 (a = (2^21-4)/max|x|).  The host ships
    mantissa bytes as u16 (m & 0xFFFF) + u8 (m >> 16); the device
    rebuilds u = m*2^-7 - 32768 = round(x*a)*2^-7 exactly in fp32 and
    splits it into two fp16 terms for the tensor engine (22-bit-exact).
    sign(conv(x, +-1)) is scale-invariant, so `a` never leaves the host.
  * All per-core inputs (u16 plane, u8 plane, 3 Toeplitz weights) are
    packed into ONE u8 blob; the device carves it up with bitcast APs.
  * Layer 1 is a Toeplitz-stationary matmul: contraction axis holds a
    10-row window x 3 channels x 2 fp16 terms (64 rows); the 3 kernel
    taps along the column axis are PSUM-accumulated matmuls with the
    rhs shifted in the free dim.
  * Layers 2/3 have +-1 x +-1 products with small integer sums: fp8
    inputs with fp32 PSUM accumulation are bit-exact, using fp8
    DoubleRow perf mode (2 MACs/cell/cycle).
  * The ternary output {-1,0,1} is packed 4 values/byte (base-4 digits)
    on the vector engine, shrinking the download 4x; the host decodes
    with a 256x4 LUT gather.
"""

import numpy as np
import ml_dtypes

import concourse.bacc as bacc
import concourse.mybir as mybir
import concourse.tile as tile
from concourse.bass_utils import run_bass_kernel_spmd

F32 = mybir.dt.float32
F16 = mybir.dt.float16
F8 = mybir.dt.float8e4
U16 = mybir.dt.uint16
U8 = mybir.dt.uint8
NP_F8 = ml_dtypes.float8_e4m3
ALU = mybir.AluOpType
DR = mybir.MatmulPerfMode.DoubleRow
SIGN = mybir.ActivationFunctionType.Sign

N_CORES = 8

AL1, AO1 = 10, 8     # L1: rows window / rows out per block
AL2, AO2 = 7, 5      # L2
AL3, AO3 = 32, 30    # L3
C1, C2, C3 = 3, 16, 23
O1, O2, O3 = 16, 23, 2

MAGIC = np.float32(3 * 2.0**22)   # forces t into [2^23, 2^24): ulp == 1
PAD_HI = 64                       # (2^22 >> 16): byte-2 code of x == 0
PAD_LO = 0

T1_B = 64 * 384 * 2
T2_B = 56 * 768
T3_B = 128 * 1152

# base-4 trit decode LUT: byte -> 4 values in {-1,0,1} (garbage digit 3 -> 2)
_LUT = np.empty((256, 4), np.float32)
for _b in range(256):
    for _i in range(4):
        _LUT[_b, _i] = ((_b >> (2 * _i)) & 3) - 1


def _dims(n_img, A, B):
    nblk1 = -(-(A - 2) // AO1)
    a_pad = 16 * (-(-nblk1 // 2)) + 2
    lo_b = n_img * 3 * a_pad * B * 2
    hi_b = n_img * 3 * a_pad * B
    tot = lo_b + hi_b + T1_B + T2_B + T3_B
    return nblk1, a_pad, lo_b, hi_b, tot


def _toeplitz_weights(w1, w2, w3):
    """Build the stationary Toeplitz matrices (host side)."""
    s1 = np.sign(w1).astype(np.float32)  # [16,3,3,3]
    s2 = np.sign(w2).astype(np.float32)  # [23,16,3,3]
    s3 = np.sign(w3).astype(np.float32)  # [2,23,3,3]

    # T1[(term*32 + al*3 + c), dx, (aol*16 + o)]; spacer rows zero.
    # term 0 multiplies the fp16 hi part, term 1 the fp16 mid part.
    t1 = np.zeros((64, 3, AO1 * O1), np.float32)
    for al in range(AL1):
        for aol in range(AO1):
            dy = al - aol
            if 0 <= dy <= 2:
                for c in range(C1):
                    for t in range(2):
                        for o in range(O1):
                            t1[t * 32 + al * 3 + c, :, aol * 16 + o] = s1[o, c, dy, :]
    # T2[(al*8 + cp), dx, codd, (aol*23 + o)] fp8 DoubleRow pairs, M pad 128
    t2 = np.zeros((56, 3, 2, 128), np.float32)
    for al in range(AL2):
        for aol in range(AO2):
            dy = al - aol
            if 0 <= dy <= 2:
                for c in range(C2):
                    for o in range(O2):
                        t2[al * 8 + c // 2, :, c % 2, aol * 23 + o] = s2[o, c, dy, :]
    # T3[(al*4 + cp), cc, dx, codd, (o*30 + aol)] fp8 DoubleRow, M pad 64
    t3 = np.zeros((128, 3, 3, 2, 64), np.float32)
    for al in range(AL3):
        for aol in range(AO3):
            dy = al - aol
            if 0 <= dy <= 2:
                for cc in range(3):
                    for cl in range(8):
                        c = cc * 8 + cl
                        if c < C3:
                            for o in range(O3):
                                t3[al * 4 + cl // 2, cc, :, cl % 2, o * AO3 + aol] = (
                                    s3[o, c, dy, :]
                                )
    return (
        t1.reshape(64, 3 * 128).astype(np.float16),
        t2.reshape(56, 3 * 256).astype(NP_F8),
        t3.reshape(128, 9 * 128).astype(NP_F8),
    )


def _build_program(n_img, A, B, layers=(1, 2, 3)):
    """Emit the per-core SPMD Bass program (all 3 layers, n_img images)."""
    nblk1, a_pad, lo_b, hi_b, tot = _dims(n_img, A, B)
    a1 = AO1 * nblk1                    # s1 row count (incl. garbage tail)
    nblk2 = -(-(A - 4) // AO2)
    nblk3 = -(-(A - 6) // AO3)
    s2a = max(AO2 * nblk2, AO3 * (nblk3 - 1) + AL3)  # s2 rows incl. zero pad
    n1, n2, n3 = B - 2, B - 4, B - 6
    a3 = A - 6                          # valid output rows
    ng = n3 // 4 + 1                    # packed byte groups per row

    assert AL2 + AO2 * (nblk2 - 1) <= a1, "L2 reads past s1"

    nc = bacc.Bacc("TRN2", target_bir_lowering=False, debug=False)

    blob = nc.dram_tensor("blob", [tot], U8, kind="ExternalInput")
    bap = blob.ap()
    xlo = bap[0:lo_b].bitcast(U16).rearrange(
        "(n c h w) -> n c h w", n=n_img, c=3, h=a_pad
    )
    xhi = bap[lo_b : lo_b + hi_b].rearrange(
        "(n c h w) -> n c h w", n=n_img, c=3, h=a_pad
    )
    o1 = lo_b + hi_b
    t1w = bap[o1 : o1 + T1_B].bitcast(F16).rearrange("(p m) -> p m", p=64)
    t2w = bap[o1 + T1_B : o1 + T1_B + T2_B].bitcast(F8).rearrange(
        "(p m) -> p m", p=56
    )
    t3w = bap[o1 + T1_B + T2_B : o1 + T1_B + T2_B + T3_B].bitcast(F8).rearrange(
        "(p m) -> p m", p=128
    )
    outp = nc.dram_tensor("outp", [n_img, 2, a3, ng], U8, kind="ExternalOutput")
    s1d = [
        nc.dram_tensor(f"s1_{i}", [a1, 16, n1], F8, kind="Internal")
        for i in range(n_img)
    ]
    s2d = [
        nc.dram_tensor(f"s2_{i}", [s2a, 24, n2], F8, kind="Internal")
        for i in range(n_img)
    ]

    with tile.TileContext(nc) as tc:
        with (
            tc.tile_pool(name="const", bufs=1) as cpool,
            tc.tile_pool(name="l1x", bufs=2) as p1x,
            tc.tile_pool(name="l1s", bufs=4) as p1s,
            tc.tile_pool(name="l2", bufs=4) as p2,
            tc.tile_pool(name="l3", bufs=8) as p3,
            tc.tile_pool(name="ps1", bufs=3, space="PSUM") as ps1p,
            tc.tile_pool(name="ps2", bufs=2, space="PSUM") as ps2p,
            tc.tile_pool(name="ps3", bufs=2, space="PSUM") as ps3p,
        ):
            t1sb = cpool.tile([64, 3 * 128], F16)
            t2sb = cpool.tile([56, 3 * 256], F8)
            t3sb = cpool.tile([128, 9 * 128], F8)
            ztile = cpool.tile([128, B], F8)
            nc.sync.dma_start(t1sb[:], t1w)
            nc.sync.dma_start(t2sb[:], t2w)
            nc.sync.dma_start(t3sb[:], t3w)
            nc.vector.memset(ztile[:], 0.0)
            # persistent L1 rhs ring: hi rows 0..29, mid rows 32..61.
            NRHS = 3
            rhs_ring = []
            for ri in range(NRHS):
                rt = cpool.tile([64, B], F16, name=f"rhs1_{ri}")
                nc.vector.memset(rt[:], 0.0)
                rhs_ring.append(rt)

            for img in range(n_img):
                s1, s2 = s1d[img].ap(), s2d[img].ap()
                # ---- zero pads of s2: channel-23 plane + tail rows ----
                for r in range(0, s2a, 128):
                    cnt = min(128, s2a - r)
                    nc.sync.dma_start(s2[r : r + cnt, 23, :], ztile[:cnt, :n2])
                for a in range(AO2 * nblk2, s2a):
                    nc.sync.dma_start(s2[a, :, :], ztile[:24, :n2])

                # ---------------- layer 1 ----------------
                # row-groups of up to 15 blocks; u16/u8 -> exact fp32
                # fixed-point value -> 2-way fp16 split runs once per
                # group with rows on partitions, then per-block rhs
                # assembly is pure SBUF->SBUF DMA.
                for g0 in range(0, nblk1 if 1 in layers else 0, 15):
                    g1 = min(g0 + 15, nblk1)
                    r0 = 8 * g0
                    rcnt = 8 * (g1 - 1 - g0) + AL1
                    lo_t = p1x.tile([122, 3 * B], U16, tag="lo")
                    hi_t = p1x.tile([122, 3 * B], U8, tag="hi")
                    nc.sync.dma_start(
                        lo_t[0:rcnt, :],
                        xlo[img, :, r0 : r0 + rcnt, :].transpose([1, 0, 2]),
                    )
                    nc.sync.dma_start(
                        hi_t[0:rcnt, :],
                        xhi[img, :, r0 : r0 + rcnt, :].transpose([1, 0, 2]),
                    )
                    lo32 = p1x.tile([122, 3 * B], F32, tag="lo32")
                    hi32 = p1x.tile([122, 3 * B], F32, tag="hi32")
                    nc.vector.tensor_copy(lo32[0:rcnt, :], lo_t[0:rcnt, :])
                    nc.vector.tensor_copy(hi32[0:rcnt, :], hi_t[0:rcnt, :])
                    # u = hi*512 - 32768 + lo*2^-7  (exact in fp32)
                    ug = p1x.tile([122, 3 * B], F32, tag="ug")
                    nc.vector.tensor_scalar(
                        hi32[0:rcnt, :], hi32[0:rcnt, :], 512.0, -32768.0,
                        op0=ALU.mult, op1=ALU.add,
                    )
                    nc.vector.scalar_tensor_tensor(
                        ug[0:rcnt, :], lo32[0:rcnt, :], 2.0**-7, hi32[0:rcnt, :],
                        op0=ALU.mult, op1=ALU.add,
                    )
                    hig = p1x.tile([122, 3 * B], F16, tag="hig")
                    dg = p1x.tile([122, 3 * B], F32, tag="dg")
                    mig = p1x.tile([122, 3 * B], F16, tag="mig")
                    nc.vector.tensor_copy(hig[0:rcnt, :], ug[0:rcnt, :])
                    nc.vector.scalar_tensor_tensor(
                        dg[0:rcnt, :], ug[0:rcnt, :], 1.0, hig[0:rcnt, :],
                        op0=ALU.mult, op1=ALU.subtract,
                    )
                    nc.vector.tensor_copy(mig[0:rcnt, :], dg[0:rcnt, :])
                    for blk in range(g0, g1):
                        a0 = 8 * blk
                        lr = a0 - r0
                        rhs16 = rhs_ring[blk % NRHS]
                        for ti, tsrc in enumerate((hig, mig)):
                            nc.sync.dma_start(
                                rhs16[32 * ti : 32 * ti + 30, :],
                                tsrc[lr : lr + AL1, :],
                            )
                        ps = ps1p.tile([128, n1], F32, tag="ps1")
                        for dx in range(3):
                            nc.tensor.matmul(
                                ps[:],
                                t1sb[:, 128 * dx : 128 * dx + 128],
                                rhs16[:, dx : dx + n1],
                                start=(dx == 0),
                                stop=(dx == 2),
                            )
                        pos16 = p1s.tile([128, n1], F16, tag="pos16")
                        nc.vector.tensor_scalar(
                            pos16[:], ps[:], 0.0, None, op0=ALU.is_gt
                        )
                        sg8 = p1s.tile([128, n1], F8, tag="sg8")
                        nc.vector.tensor_scalar(
                            sg8[:], pos16[:], 2.0, -1.0, op0=ALU.mult, op1=ALU.add
                        )
                        nc.sync.dma_start(s1[a0 : a0 + 8, :, :], sg8[:])

                # ---------------- layer 2 (fp8 DoubleRow) ----------------
                for b in range(nblk2 if 2 in layers else 0):
                    rhs8 = p2.tile([56, 2 * 512], F8, tag="rhs8")
                    r3 = rhs8[:].rearrange("k (t h) -> k t h", t=2)
                    nc.sync.dma_start(r3[:, :, 0:n1], s1[5 * b : 5 * b + 7, :, :])
                    ps = ps2p.tile([115, n2], F32, tag="ps2")
                    for dx in range(3):
                        nc.tensor.matmul(
                            ps[:],
                            t2sb[:, 256 * dx : 256 * dx + 256].rearrange(
                                "k (t m) -> k t m", t=2
                            )[:, :, 0:115],
                            r3[:, :, dx : dx + n2],
                            start=(dx == 0),
                            stop=(dx == 2),
                            perf_mode=DR,
                        )
                    sg2 = p2.tile([115, n2], F8, tag="sg2")
                    nc.scalar.activation(sg2[:], ps[:], SIGN)
                    nc.sync.dma_start(s2[5 * b : 5 * b + 5, 0:23, :], sg2[:])

                # ---------------- layer 3 (fp8 DoubleRow) ----------------
                for bb in range(nblk3 if 3 in layers else 0):
                    rb0 = 30 * bb
                    rows = min(30, a3 - rb0)   # valid out rows this block
                    rts = []
                    for cc in range(3):
                        rt = p3.tile([128, 2 * 512], F8, tag="rhs3")
                        nc.sync.dma_start(
                            rt[:].rearrange("k (t h) -> k t h", t=2)[:, :, 0:n2],
                            s2[rb0 : rb0 + 32, 8 * cc : 8 * cc + 8, :],
                        )
                        rts.append(rt)
                    ps = ps3p.tile([60, n3], F32, tag="ps3")
                    for cc in range(3):
                        for dx in range(3):
                            nc.tensor.matmul(
                                ps[:],
                                t3sb[
                                    :, 128 * (cc * 3 + dx) : 128 * (cc * 3 + dx) + 128
                                ].rearrange("k (t m) -> k t m", t=2)[:, :, 0:60],
                                rts[cc][:].rearrange("k (t h) -> k t h", t=2)[
                                    :, :, dx : dx + n3
                                ],
                                start=(cc == 0 and dx == 0),
                                stop=(cc == 2 and dx == 2),
                                perf_mode=DR,
                            )
                    # sign -> {-1,0,1} fp16, then pack 4 cols/byte (base 4)
                    oc = p3.tile([60, 512], F16, tag="oc")
                    nc.scalar.activation(oc[:, 0:n3], ps[:], SIGN)
                    nc.vector.memset(oc[:, n3 : 4 * ng], 0.0)
                    gv = oc[:].rearrange("p (g i) -> p g i", i=4)
                    q1 = p3.tile([60, 128], F16, tag="q1")
                    q2 = p3.tile([60, 128], F16, tag="q2")
                    pk = p3.tile([60, 128], F16, tag="pk")
                    pku = p3.tile([60, 128], U8, tag="pku")
                    nc.vector.scalar_tensor_tensor(
                        q1[:, 0:ng], gv[:, 0:ng, 1], 4.0, gv[:, 0:ng, 0],
                        op0=ALU.mult, op1=ALU.add,
                    )
                    nc.vector.scalar_tensor_tensor(
                        q2[:, 0:ng], gv[:, 0:ng, 3], 4.0, gv[:, 0:ng, 2],
                        op0=ALU.mult, op1=ALU.add,
                    )
                    nc.vector.scalar_tensor_tensor(
                        pk[:, 0:ng], q2[:, 0:ng], 16.0, q1[:, 0:ng],
                        op0=ALU.mult, op1=ALU.add,
                    )
                    nc.vector.tensor_scalar(
                        pku[:, 0:ng], pk[:, 0:ng], 85.0, None, op0=ALU.add
                    )
                    for o in range(2):
                        nc.sync.dma_start(
                            outp.ap()[img, o, rb0 : rb0 + rows, :],
                            pku[AO3 * o : AO3 * o + rows, 0:ng],
                        )

    nc.compile()
    return nc


_CACHE = {}


def _get_program(n_img, A, B):
    key = (n_img, A, B)
    if key not in _CACHE:
        _CACHE[key] = _build_program(n_img, A, B)
    return _CACHE[key]


def make_blobs(x, w1, w2, w3, n_cores=N_CORES):
    """x: [N,3,A,B] fp32 -> [n_cores, tot] u8 blob (fixed-point code +
    Toeplitz weights, laid out for the device's bitcast APs)."""
    x = np.asarray(x, np.float32)
    n, _, A, B = x.shape
    per = n // n_cores
    _, a_pad, lo_b, hi_b, tot = _dims(per, A, B)
    t1, t2, t3 = _toeplitz_weights(
        np.asarray(w1, np.float32), np.asarray(w2, np.float32),
        np.asarray(w3, np.float32),
    )
    m = max(float(x.max()), -float(x.min()), 1e-30)
    a = np.float32((2.0**21 - 4) / m)
    t = x * a
    t += MAGIC
    bv = t.view(np.uint8).reshape(n, 3, A, B, 4)

    blob = np.empty((n_cores, tot), np.uint8)
    o1 = lo_b + hi_b
    wbytes = np.concatenate(
        [t1.view(np.uint8).ravel(), t2.view(np.uint8).ravel(),
         t3.view(np.uint8).ravel()]
    )
    for i in range(n_cores):
        lo = blob[i, 0:lo_b].view(np.uint16).reshape(per, 3, a_pad, B)
        lo.view(np.uint8).reshape(per, 3, a_pad, B, 2)[:, :, :A] = (
            bv[per * i : per * (i + 1), :, :, :, 0:2]
        )
        lo[:, :, A:] = PAD_LO
        hi = blob[i, lo_b:o1].reshape(per, 3, a_pad, B)
        hi[:, :, :A] = bv[per * i : per * (i + 1), :, :, :, 2]
        hi[:, :, A:] = PAD_HI
        blob[i, o1 : o1 + wbytes.size] = wbytes
    return blob


last_results = None


def kernel(inputs, w1, w2, w3):
    global last_results
    x = np.asarray(inputs, np.float32)
    n, _, A, B = x.shape
    per = n // N_CORES
    nc = _get_program(per, A, B)
    blob = make_blobs(x, w1, w2, w3)
    maps = [{"blob": blob[i]} for i in range(N_CORES)]
    res = run_bass_kernel_spmd(nc, maps, core_ids=list(range(N_CORES)))
    last_results = res
    a3, b3 = A - 6, B - 6
    out = np.empty((n, 2, a3, b3), np.float32)
    for i, r in enumerate(res.results):
        v = _LUT[r["outp"]]                       # [per, 2, a3, ng, 4]
        out[per * i : per * (i + 1)] = v.reshape(per, 2, a3, -1)[..., :b3]
    return out.reshape(n, -1)


# revision 6
# speedup vs baseline: 1.6676x; 1.5786x over previous
"""Trainium2 Bass kernel for a 3-layer binarized CNN.

Network (reference):
    x  : [32, 3, 512, 512] fp32
    l1 : clip(conv(x, sign(w1)))            -> [32,16,510,510]
    l2 : clip(conv(sign(l1), sign(w2)))     -> [32,23,508,508]
    l3 : clip(conv(sign(l2), sign(w3)))     -> [32,2,506,506]
    out: l3.reshape(32, -1)

The end-to-end wall time is dominated by the ~70-80 MB/s axon tunnel
(plus ~50-80 ms fixed cost per transferred array), so the kernel
minimizes both bytes and array count on the wire:

  * Input is sent as an exact 21-bit fixed-point code (3 B/elem instead
    of 4): t = x*a + 3*2^22 in fp32 makes the mantissa a fixed-point
    integer m = round(x*a) + 2^22 (a = (2^21-4)/max|x|).  The host ships
    mantissa bytes as u16 (m & 0xFFFF) + u8 (m >> 16); the device
    rebuilds u = m*2^-7 - 32768 = round(x*a)*2^-7 exactly in fp32 and
    splits it into two fp16 terms for the tensor engine (22-bit-exact).
    sign(conv(x, +-1)) is scale-invariant, so `a` never leaves the host.
  * All per-core inputs (u16 plane, u8 plane, 3 Toeplitz weights) are
    packed into ONE u8 blob; the device carves it up with bitcast APs.
  * Layer 1 is a Toeplitz-stationary matmul: contraction axis holds a
    10-row window x 3 channels x 2 fp16 terms (64 rows); the 3 kernel
    taps along the column axis are PSUM-accumulated matmuls with the
    rhs shifted in the free dim.
  * Layers 2/3 have +-1 x +-1 products with small integer sums: fp8
    inputs with fp32 PSUM accumulation are bit-exact, using fp8
    DoubleRow perf mode (2 MACs/cell/cycle).
  * The ternary output {-1,0,1} is packed 4 values/byte (base-4 digits)
    on the vector engine, shrinking the download 4x; the host decodes
    with a 256x4 LUT gather.
"""

import numpy as np
import ml_dtypes

import concourse.bacc as bacc
import concourse.bass2jax as _b2j
import concourse.mybir as mybir
import concourse.tile as tile
from concourse.bass_utils import run_bass_kernel_spmd

F32 = mybir.dt.float32
F16 = mybir.dt.float16
F8 = mybir.dt.float8e4
U16 = mybir.dt.uint16
U8 = mybir.dt.uint8
NP_F8 = ml_dtypes.float8_e4m3
ALU = mybir.AluOpType
DR = mybir.MatmulPerfMode.DoubleRow
SIGN = mybir.ActivationFunctionType.Sign

N_CORES = 8

AL1, AO1 = 10, 8     # L1: rows window / rows out per block
AL2, AO2 = 7, 5      # L2
AL3, AO3 = 32, 30    # L3
C1, C2, C3 = 3, 16, 23
O1, O2, O3 = 16, 23, 2

MAGIC = np.float32(3 * 2.0**22)   # forces t into [2^23, 2^24): ulp == 1
PAD_HI = 64                       # (2^22 >> 16): byte-2 code of x == 0
PAD_LO = 0

T1_B = 64 * 384 * 2
T2_B = 56 * 768
T3_B = 128 * 1152

# base-4 trit decode LUT: byte -> 4 values in {-1,0,1} (garbage digit 3 -> 2)
_LUT = np.empty((256, 4), np.float32)
for _b in range(256):
    for _i in range(4):
        _LUT[_b, _i] = ((_b >> (2 * _i)) & 3) - 1


def _dims(n_img, A, B):
    nblk1 = -(-(A - 2) // AO1)
    a_pad = 16 * (-(-nblk1 // 2)) + 2
    lo_b = n_img * 3 * a_pad * B * 2
    hi_b = n_img * 3 * a_pad * B
    tot = lo_b + hi_b + T1_B + T2_B + T3_B
    return nblk1, a_pad, lo_b, hi_b, tot


def _toeplitz_weights(w1, w2, w3):
    """Build the stationary Toeplitz matrices (host side)."""
    s1 = np.sign(w1).astype(np.float32)  # [16,3,3,3]
    s2 = np.sign(w2).astype(np.float32)  # [23,16,3,3]
    s3 = np.sign(w3).astype(np.float32)  # [2,23,3,3]

    # T1[(term*32 + al*3 + c), dx, (aol*16 + o)]; spacer rows zero.
    # term 0 multiplies the fp16 hi part, term 1 the fp16 mid part.
    t1 = np.zeros((64, 3, AO1 * O1), np.float32)
    for al in range(AL1):
        for aol in range(AO1):
            dy = al - aol
            if 0 <= dy <= 2:
                for c in range(C1):
                    for t in range(2):
                        for o in range(O1):
                            t1[t * 32 + al * 3 + c, :, aol * 16 + o] = s1[o, c, dy, :]
    # T2[(al*8 + cp), dx, codd, (aol*23 + o)] fp8 DoubleRow pairs, M pad 128
    t2 = np.zeros((56, 3, 2, 128), np.float32)
    for al in range(AL2):
        for aol in range(AO2):
            dy = al - aol
            if 0 <= dy <= 2:
                for c in range(C2):
                    for o in range(O2):
                        t2[al * 8 + c // 2, :, c % 2, aol * 23 + o] = s2[o, c, dy, :]
    # T3[(al*4 + cp), cc, dx, codd, (o*30 + aol)] fp8 DoubleRow, M pad 64
    t3 = np.zeros((128, 3, 3, 2, 64), np.float32)
    for al in range(AL3):
        for aol in range(AO3):
            dy = al - aol
            if 0 <= dy <= 2:
                for cc in range(3):
                    for cl in range(8):
                        c = cc * 8 + cl
                        if c < C3:
                            for o in range(O3):
                                t3[al * 4 + cl // 2, cc, :, cl % 2, o * AO3 + aol] = (
                                    s3[o, c, dy, :]
                                )
    return (
        t1.reshape(64, 3 * 128).astype(np.float16),
        t2.reshape(56, 3 * 256).astype(NP_F8),
        t3.reshape(128, 9 * 128).astype(NP_F8),
    )


def _build_program(n_img, A, B, layers=(1, 2, 3)):
    """Emit the per-core SPMD Bass program (all 3 layers, n_img images)."""
    nblk1, a_pad, lo_b, hi_b, tot = _dims(n_img, A, B)
    a1 = AO1 * nblk1                    # s1 row count (incl. garbage tail)
    nblk2 = -(-(A - 4) // AO2)
    nblk3 = -(-(A - 6) // AO3)
    s2a = max(AO2 * nblk2, AO3 * (nblk3 - 1) + AL3)  # s2 rows incl. zero pad
    n1, n2, n3 = B - 2, B - 4, B - 6
    a3 = A - 6                          # valid output rows
    ng = n3 // 4 + 1                    # packed byte groups per row

    assert AL2 + AO2 * (nblk2 - 1) <= a1, "L2 reads past s1"

    nc = bacc.Bacc("TRN2", target_bir_lowering=False, debug=False)

    blob = nc.dram_tensor("blob", [tot], U8, kind="ExternalInput")
    bap = blob.ap()
    xlo = bap[0:lo_b].bitcast(U16).rearrange(
        "(n c h w) -> n c h w", n=n_img, c=3, h=a_pad
    )
    xhi = bap[lo_b : lo_b + hi_b].rearrange(
        "(n c h w) -> n c h w", n=n_img, c=3, h=a_pad
    )
    o1 = lo_b + hi_b
    t1w = bap[o1 : o1 + T1_B].bitcast(F16).rearrange("(p m) -> p m", p=64)
    t2w = bap[o1 + T1_B : o1 + T1_B + T2_B].bitcast(F8).rearrange(
        "(p m) -> p m", p=56
    )
    t3w = bap[o1 + T1_B + T2_B : o1 + T1_B + T2_B + T3_B].bitcast(F8).rearrange(
        "(p m) -> p m", p=128
    )
    outp = nc.dram_tensor("outp", [n_img, 2, a3, ng], U8, kind="ExternalOutput")
    s1d = [
        nc.dram_tensor(f"s1_{i}", [a1, 16, n1], F8, kind="Internal")
        for i in range(n_img)
    ]
    s2d = [
        nc.dram_tensor(f"s2_{i}", [s2a, 24, n2], F8, kind="Internal")
        for i in range(n_img)
    ]

    with tile.TileContext(nc) as tc:
        with (
            tc.tile_pool(name="const", bufs=1) as cpool,
            tc.tile_pool(name="l1x", bufs=2) as p1x,
            tc.tile_pool(name="l1s", bufs=4) as p1s,
            tc.tile_pool(name="l2", bufs=4) as p2,
            tc.tile_pool(name="l3", bufs=8) as p3,
            tc.tile_pool(name="ps1", bufs=3, space="PSUM") as ps1p,
            tc.tile_pool(name="ps2", bufs=2, space="PSUM") as ps2p,
            tc.tile_pool(name="ps3", bufs=2, space="PSUM") as ps3p,
        ):
            t1sb = cpool.tile([64, 3 * 128], F16)
            t2sb = cpool.tile([56, 3 * 256], F8)
            t3sb = cpool.tile([128, 9 * 128], F8)
            ztile = cpool.tile([128, B], F8)
            nc.sync.dma_start(t1sb[:], t1w)
            nc.sync.dma_start(t2sb[:], t2w)
            nc.sync.dma_start(t3sb[:], t3w)
            nc.vector.memset(ztile[:], 0.0)
            # persistent L1 rhs ring: hi rows 0..29, mid rows 32..61.
            NRHS = 3
            rhs_ring = []
            for ri in range(NRHS):
                rt = cpool.tile([64, B], F16, name=f"rhs1_{ri}")
                nc.vector.memset(rt[:], 0.0)
                rhs_ring.append(rt)

            for img in range(n_img):
                s1, s2 = s1d[img].ap(), s2d[img].ap()
                # ---- zero pads of s2: channel-23 plane + tail rows ----
                for r in range(0, s2a, 128):
                    cnt = min(128, s2a - r)
                    nc.sync.dma_start(s2[r : r + cnt, 23, :], ztile[:cnt, :n2])
                for a in range(AO2 * nblk2, s2a):
                    nc.sync.dma_start(s2[a, :, :], ztile[:24, :n2])

                # ---------------- layer 1 ----------------
                # row-groups of up to 15 blocks; u16/u8 -> exact fp32
                # fixed-point value -> 2-way fp16 split runs once per
                # group with rows on partitions, then per-block rhs
                # assembly is pure SBUF->SBUF DMA.
                for g0 in range(0, nblk1 if 1 in layers else 0, 15):
                    g1 = min(g0 + 15, nblk1)
                    r0 = 8 * g0
                    rcnt = 8 * (g1 - 1 - g0) + AL1
                    lo_t = p1x.tile([122, 3 * B], U16, tag="lo")
                    hi_t = p1x.tile([122, 3 * B], U8, tag="hi")
                    nc.sync.dma_start(
                        lo_t[0:rcnt, :],
                        xlo[img, :, r0 : r0 + rcnt, :].transpose([1, 0, 2]),
                    )
                    nc.sync.dma_start(
                        hi_t[0:rcnt, :],
                        xhi[img, :, r0 : r0 + rcnt, :].transpose([1, 0, 2]),
                    )
                    lo32 = p1x.tile([122, 3 * B], F32, tag="lo32")
                    hi32 = p1x.tile([122, 3 * B], F32, tag="hi32")
                    nc.vector.tensor_copy(lo32[0:rcnt, :], lo_t[0:rcnt, :])
                    nc.vector.tensor_copy(hi32[0:rcnt, :], hi_t[0:rcnt, :])
                    # u = hi*512 - 32768 + lo*2^-7  (exact in fp32)
                    ug = p1x.tile([122, 3 * B], F32, tag="ug")
                    nc.vector.tensor_scalar(
                        hi32[0:rcnt, :], hi32[0:rcnt, :], 512.0, -32768.0,
                        op0=ALU.mult, op1=ALU.add,
                    )
                    nc.vector.scalar_tensor_tensor(
                        ug[0:rcnt, :], lo32[0:rcnt, :], 2.0**-7, hi32[0:rcnt, :],
                        op0=ALU.mult, op1=ALU.add,
                    )
                    hig = p1x.tile([122, 3 * B], F16, tag="hig")
                    dg = p1x.tile([122, 3 * B], F32, tag="dg")
                    mig = p1x.tile([122, 3 * B], F16, tag="mig")
                    nc.vector.tensor_copy(hig[0:rcnt, :], ug[0:rcnt, :])
                    nc.vector.scalar_tensor_tensor(
                        dg[0:rcnt, :], ug[0:rcnt, :], 1.0, hig[0:rcnt, :],
                        op0=ALU.mult, op1=ALU.subtract,
                    )
                    nc.vector.tensor_copy(mig[0:rcnt, :], dg[0:rcnt, :])
                    for blk in range(g0, g1):
                        a0 = 8 * blk
                        lr = a0 - r0
                        rhs16 = rhs_ring[blk % NRHS]
                        for ti, tsrc in enumerate((hig, mig)):
                            nc.sync.dma_start(
                                rhs16[32 * ti : 32 * ti + 30, :],
                                tsrc[lr : lr + AL1, :],
                            )
                        ps = ps1p.tile([128, n1], F32, tag="ps1")
                        for dx in range(3):
                            nc.tensor.matmul(
                                ps[:],
                                t1sb[:, 128 * dx : 128 * dx + 128],
                                rhs16[:, dx : dx + n1],
                                start=(dx == 0),
                                stop=(dx == 2),
                            )
                        pos16 = p1s.tile([128, n1], F16, tag="pos16")
                        nc.vector.tensor_scalar(
                            pos16[:], ps[:], 0.0, None, op0=ALU.is_gt
                        )
                        sg8 = p1s.tile([128, n1], F8, tag="sg8")
                        nc.vector.tensor_scalar(
                            sg8[:], pos16[:], 2.0, -1.0, op0=ALU.mult, op1=ALU.add
                        )
                        nc.sync.dma_start(s1[a0 : a0 + 8, :, :], sg8[:])

                # ---------------- layer 2 (fp8 DoubleRow) ----------------
                for b in range(nblk2 if 2 in layers else 0):
                    rhs8 = p2.tile([56, 2 * 512], F8, tag="rhs8")
                    r3 = rhs8[:].rearrange("k (t h) -> k t h", t=2)
                    nc.sync.dma_start(r3[:, :, 0:n1], s1[5 * b : 5 * b + 7, :, :])
                    ps = ps2p.tile([115, n2], F32, tag="ps2")
                    for dx in range(3):
                        nc.tensor.matmul(
                            ps[:],
                            t2sb[:, 256 * dx : 256 * dx + 256].rearrange(
                                "k (t m) -> k t m", t=2
                            )[:, :, 0:115],
                            r3[:, :, dx : dx + n2],
                            start=(dx == 0),
                            stop=(dx == 2),
                            perf_mode=DR,
                        )
                    sg2 = p2.tile([115, n2], F8, tag="sg2")
                    nc.scalar.activation(sg2[:], ps[:], SIGN)
                    nc.sync.dma_start(s2[5 * b : 5 * b + 5, 0:23, :], sg2[:])

                # ---------------- layer 3 (fp8 DoubleRow) ----------------
                for bb in range(nblk3 if 3 in layers else 0):
                    rb0 = 30 * bb
                    rows = min(30, a3 - rb0)   # valid out rows this block
                    rts = []
                    for cc in range(3):
                        rt = p3.tile([128, 2 * 512], F8, tag="rhs3")
                        nc.sync.dma_start(
                            rt[:].rearrange("k (t h) -> k t h", t=2)[:, :, 0:n2],
                            s2[rb0 : rb0 + 32, 8 * cc : 8 * cc + 8, :],
                        )
                        rts.append(rt)
                    ps = ps3p.tile([60, n3], F32, tag="ps3")
                    for cc in range(3):
                        for dx in range(3):
                            nc.tensor.matmul(
                                ps[:],
                                t3sb[
                                    :, 128 * (cc * 3 + dx) : 128 * (cc * 3 + dx) + 128
                                ].rearrange("k (t m) -> k t m", t=2)[:, :, 0:60],
                                rts[cc][:].rearrange("k (t h) -> k t h", t=2)[
                                    :, :, dx : dx + n3
                                ],
                                start=(cc == 0 and dx == 0),
                                stop=(cc == 2 and dx == 2),
                                perf_mode=DR,
                            )
                    # sign -> {-1,0,1} fp16, then pack 4 cols/byte (base 4)
                    oc = p3.tile([60, 512], F16, tag="oc")
                    nc.scalar.activation(oc[:, 0:n3], ps[:], SIGN)
                    nc.vector.memset(oc[:, n3 : 4 * ng], 0.0)
                    gv = oc[:].rearrange("p (g i) -> p g i", i=4)
                    q1 = p3.tile([60, 128], F16, tag="q1")
                    q2 = p3.tile([60, 128], F16, tag="q2")
                    pk = p3.tile([60, 128], F16, tag="pk")
                    pku = p3.tile([60, 128], U8, tag="pku")
                    nc.vector.scalar_tensor_tensor(
                        q1[:, 0:ng], gv[:, 0:ng, 1], 4.0, gv[:, 0:ng, 0],
                        op0=ALU.mult, op1=ALU.add,
                    )
                    nc.vector.scalar_tensor_tensor(
                        q2[:, 0:ng], gv[:, 0:ng, 3], 4.0, gv[:, 0:ng, 2],
                        op0=ALU.mult, op1=ALU.add,
                    )
                    nc.vector.scalar_tensor_tensor(
                        pk[:, 0:ng], q2[:, 0:ng], 16.0, q1[:, 0:ng],
                        op0=ALU.mult, op1=ALU.add,
                    )
                    nc.vector.tensor_scalar(
                        pku[:, 0:ng], pk[:, 0:ng], 85.0, None, op0=ALU.add
                    )
                    for o in range(2):
                        nc.sync.dma_start(
                            outp.ap()[img, o, rb0 : rb0 + rows, :],
                            pku[AO3 * o : AO3 * o + rows, 0:ng],
                        )

    nc.compile()
    return nc


_CACHE = {}


def _get_program(n_img, A, B):
    key = (n_img, A, B)
    if key not in _CACHE:
        _CACHE[key] = _build_program(n_img, A, B)
    return _CACHE[key]


# ---------------------------------------------------------------------------
# Stock bass2jax.run_bass_via_pjrt rebuilds its jax.jit closure on every call,
# which forces a full shard_map retrace (~0.5 s) per invocation.  Functionally
# identical replacement that caches the jitted callable per (nc, n_cores).
# ---------------------------------------------------------------------------
_JIT_CACHE = {}


def _cached_run_bass_via_pjrt(nc, in_maps, n_cores):
    import jax
    from jax.experimental.shard_map import shard_map
    from jax.sharding import Mesh, PartitionSpec

    key = (id(nc), n_cores)
    if key not in _JIT_CACHE:
        _b2j.install_neuronx_cc_hook()
        if nc.dbg_addr is not None and nc.dbg_callbacks:
            raise RuntimeError("dbg_callbacks unsupported under axon")
        partition_name = (
            nc.partition_id_tensor.name if nc.partition_id_tensor else None
        )
        in_names, out_names, out_avals, zero_shapes = [], [], [], []
        for alloc in nc.m.functions[0].allocations:
            if not isinstance(alloc, mybir.MemoryLocationSet):
                continue
            name = alloc.memorylocations[0].name
            if alloc.kind == "ExternalInput":
                if name != partition_name and name != (
                    nc.dbg_addr.name if nc.dbg_addr else None
                ):
                    in_names.append(name)
            elif alloc.kind == "ExternalOutput":
                out_names.append(name)
                shape = tuple(alloc.tensor_shape)
                dtype = mybir.dt.np(alloc.dtype)
                out_avals.append(jax.core.ShapedArray(shape, dtype))
                zero_shapes.append((shape, dtype))
        n_params, n_outs = len(in_names), len(out_avals)
        in_names_full = list(in_names) + out_names
        if nc.dbg_addr is not None:
            in_names_full.append(nc.dbg_addr.name)
        if partition_name is not None:
            in_names_full.append(partition_name)

        def _body(*args):
            operands = list(args)
            if nc.dbg_addr is not None:
                operands.append(jax.numpy.zeros((1, 2), np.uint32))
            if partition_name is not None:
                operands.append(_b2j.partition_id_tensor())
            return tuple(
                _b2j._bass_exec_p.bind(
                    *operands,
                    out_avals=tuple(out_avals),
                    in_names=tuple(in_names_full),
                    out_names=tuple(out_names),
                    lowering_input_output_aliases=(),
                    sim_require_finite=True,
                    sim_require_nnan=True,
                    nc=nc,
                )
            )

        mesh = Mesh(np.asarray(jax.devices()[:n_cores]), ("core",))
        donate = tuple(range(n_params, n_params + n_outs))
        sharded = jax.jit(
            shard_map(
                _body,
                mesh=mesh,
                in_specs=(PartitionSpec("core"),) * (n_params + n_outs),
                out_specs=(PartitionSpec("core"),) * n_outs,
                check_rep=False,
            ),
            donate_argnums=donate,
            keep_unused=True,
        )
        _JIT_CACHE[key] = (sharded, in_names, out_names, out_avals, zero_shapes)

    sharded, in_names, out_names, out_avals, zero_shapes = _JIT_CACHE[key]
    concat_in = [
        np.concatenate([m[nm] for m in in_maps], axis=0) for nm in in_names
    ]
    concat_zeros = [
        np.zeros((n_cores * s[0], *s[1:]), d) for s, d in zero_shapes
    ]
    out_arrs = sharded(*concat_in, *concat_zeros)
    return [
        {
            name: np.asarray(out_arrs[i]).reshape(n_cores, *out_avals[i].shape)[c]
            for i, name in enumerate(out_names)
        }
        for c in range(n_cores)
    ]


_b2j.run_bass_via_pjrt = _cached_run_bass_via_pjrt


def make_blobs(x, w1, w2, w3, n_cores=N_CORES):
    """x: [N,3,A,B] fp32 -> [n_cores, tot] u8 blob (fixed-point code +
    Toeplitz weights, laid out for the device's bitcast APs)."""
    x = np.asarray(x, np.float32)
    n, _, A, B = x.shape
    per = n // n_cores
    _, a_pad, lo_b, hi_b, tot = _dims(per, A, B)
    t1, t2, t3 = _toeplitz_weights(
        np.asarray(w1, np.float32), np.asarray(w2, np.float32),
        np.asarray(w3, np.float32),
    )
    m = max(float(x.max()), -float(x.min()), 1e-30)
    a = np.float32((2.0**21 - 4) / m)
    t = x * a
    t += MAGIC
    bv = t.view(np.uint8).reshape(n, 3, A, B, 4)

    blob = np.empty((n_cores, tot), np.uint8)
    o1 = lo_b + hi_b
    wbytes = np.concatenate(
        [t1.view(np.uint8).ravel(), t2.view(np.uint8).ravel(),
         t3.view(np.uint8).ravel()]
    )
    for i in range(n_cores):
        lo = blob[i, 0:lo_b].view(np.uint16).reshape(per, 3, a_pad, B)
        lo.view(np.uint8).reshape(per, 3, a_pad, B, 2)[:, :, :A] = (
            bv[per * i : per * (i + 1), :, :, :, 0:2]
        )
        lo[:, :, A:] = PAD_LO
        hi = blob[i, lo_b:o1].reshape(per, 3, a_pad, B)
        hi[:, :, :A] = bv[per * i : per * (i + 1), :, :, :, 2]
        hi[:, :, A:] = PAD_HI
        blob[i, o1 : o1 + wbytes.size] = wbytes
    return blob


last_results = None


def kernel(inputs, w1, w2, w3):
    global last_results
    x = np.asarray(inputs, np.float32)
    n, _, A, B = x.shape
    per = n // N_CORES
    nc = _get_program(per, A, B)
    blob = make_blobs(x, w1, w2, w3)
    maps = [{"blob": blob[i]} for i in range(N_CORES)]
    res = run_bass_kernel_spmd(nc, maps, core_ids=list(range(N_CORES)))
    last_results = res
    a3, b3 = A - 6, B - 6
    out = np.empty((n, 2, a3, b3), np.float32)
    for i, r in enumerate(res.results):
        v = _LUT[r["outp"]]                       # [per, 2, a3, ng, 4]
        out[per * i : per * (i + 1)] = v.reshape(per, 2, a3, -1)[..., :b3]
    return out.reshape(n, -1)
